# revision 10
# baseline (speedup 1.0000x reference)
"""APPNP GNN kernel for 8 Trainium2 NeuronCores (Bass/Tile).

Strategy
--------
Nodes are sharded row-wise across 8 cores; edges are partitioned by the owner
of their DST node so the segment-sum scatter is core-local.

State: g = norm_out * h, stored bf16 in 256-byte rows (128 bf16, first 48
used) so the MoE dma_gather ucode (256B-aligned rows, int16 indices) can
fetch per-edge messages. Per hop:
  1. AllGather of the local shard of g (3.2MB/rank) -> gfull [100352, 128]b16
  2. dma_gather of g[src] for the core's edges. int16 indices limit one call
     to 32768 rows, so edges are split into 4 source-row-range GROUPS.
  3. segment-sum via mask matmuls: edges sorted by dst, packed into chunks of
     128 within a (64-node window x group); a host-built one-hot bf16 mask
     [128,64] is the stationary operand; PSUM accumulates a 128-node quad.
  4. per-quad combine on DVE: g' = (0.9*no*ni)*agg + 0.1*no*h0
The MLP runs once, row-sharded, bf16 with fp32 accumulation (feats
pre-transposed on host).

The chunk schedule must be IDENTICAL on all 8 cores (one SPMD program):
chunk counts per (window, group) are maxed over cores; unfilled slots get
dummy edges pointing at the group's base row with all-zero mask rows.
Windows are processed in blocks of WBLK; each block streams its gather
output, masks and indices through double-buffered SBUF tiles.

Device DRAM layouts are partition-major (node local id l = q*128 + p lives
at staging [p, q]); the host builds indices / unpermutes accordingly.
"""
import os
import sys

import numpy as np
import ml_dtypes

sys.path.insert(0, "/opt/trn_rl_repo")

os.environ.setdefault("NEURON_SCRATCHPAD_PAGE_SIZE", "384")

import concourse.bass as bass
import concourse.bacc as bacc
import concourse.tile as tile
import concourse.mybir as mybir
import concourse.bass_utils as bass_utils

# ---- problem constants (hardcoded; kernel.py must be self-contained) ----
N, E = 100000, 1600000
IN_DIM, HID_DIM, OUT_DIM = 512, 256, 48
K_HOPS = 10
ALPHA = 0.1

NC = 8             # cores
P = 128            # partitions
W = 64             # dst window width per mask chunk
ROW = 128          # bf16 elements per state row (256 bytes)
GRP = 32768        # rows per gather range-group (int16 index limit)
WBLK = 8           # windows per streaming block
MAX_CALL = 6144    # max gather positions per dma_gather (SWDGE ring limit)

BF16 = mybir.dt.bfloat16
F32 = mybir.dt.float32
I16 = mybir.dt.int16


def configure(n=100000, e=1600000, k_hops=10):
    global N, E, K_HOPS, SH, Q, SHP, NW, NROWS, NGRP
    N, E, K_HOPS = n, e, k_hops
    SH = N // NC                 # real rows per shard
    Q = (SH + P - 1) // P        # quads per shard
    SHP = Q * P                  # padded rows per shard
    NW = SHP // W                # 64-node windows per shard
    NROWS = NC * P * Q           # rows in the AllGather buffer
    NGRP = (NROWS + GRP - 1) // GRP


configure()


# ======================================================================
# host-side preprocessing
# ======================================================================
def make_plan(chunks_wg):
    """Structural schedule from per-(window, group) chunk counts.

    Returns blocks: each with global chunk range, position/idx-column range,
    per-group gather calls, and the PE matmul order.
    """
    nwin = chunks_wg.shape[0]
    blocks = []
    j = 0          # global chunk counter
    pos = 0        # global gather position counter
    for b0 in range(0, nwin, WBLK):
        ws = list(range(b0, min(b0 + WBLK, nwin)))
        chunk0, pos0 = j, pos
        calls = []
        cmap = {}  # (w, g, c) -> col within block
        col = 0
        for g in range(NGRP):
            ncg = int(sum(chunks_wg[w][g] for w in ws))
            if ncg == 0:
                continue
            calls.append(dict(g=g, pos0=pos, npos=ncg * P, col0=col))
            for w in ws:
                for c in range(chunks_wg[w][g]):
                    cmap[(w, g, c)] = col
                    col += 1
            pos += ncg * P
        pe = []
        for w in ws:
            wchunks = [(g, c) for g in range(NGRP)
                       for c in range(chunks_wg[w][g])]
            for i, (g, c) in enumerate(wchunks):
                pe.append(dict(w=w, col=cmap[(w, g, c)],
                               start=(i == 0), stop=(i == len(wchunks) - 1)))
        j = chunk0 + col
        blocks.append(dict(chunk0=chunk0, nchunks=col, pos0=pos0,
                           npos=pos - pos0, calls=calls, pe=pe, ws=ws))
    return blocks, j, pos


def preprocess(feats, src, dst, W1, b1, W2, b2):
    feats = np.asarray(feats, dtype=np.float32)
    src = np.asarray(src).astype(np.int64)
    dst = np.asarray(dst).astype(np.int64)
    W1 = np.asarray(W1, dtype=np.float32)
    W2 = np.asarray(W2, dtype=np.float32)
    b1 = np.asarray(b1, dtype=np.float32)
    b2 = np.asarray(b2, dtype=np.float32)

    deg_out = np.maximum(np.bincount(src, minlength=N), 1.0)
    deg_in = np.maximum(np.bincount(dst, minlength=N), 1.0)
    no = deg_out ** -0.5
    ni = deg_in ** -0.5

    own = src // SH
    loc = src % SH
    grow = ((own * P + loc % P) * Q + loc // P).astype(np.int64)
    ggrp = grow // GRP

    e_owner = dst // SH
    dloc = dst % SH

    # per-(core, window, group) counts -> structural chunk counts
    wg = (dloc // W) * NGRP + ggrp
    cnt = np.zeros((NC, NW * NGRP), dtype=np.int64)
    for r in range(NC):
        cnt[r] = np.bincount(wg[e_owner == r], minlength=NW * NGRP)
    chunks_wg = np.ceil(cnt / P).astype(np.int64).max(axis=0).reshape(NW, NGRP)
    empty_w = chunks_wg.sum(axis=1) == 0
    chunks_wg[empty_w, 0] = 1

    blocks, n_chunks, n_pos = make_plan(chunks_wg)

    # chunk slot tables: global chunk id and gather-position base per chunk,
    # assigned exactly as make_plan assigned columns
    chunk_pos = np.zeros(n_chunks, dtype=np.int64)
    chunk_id = {}
    for blk in blocks:
        col = 0
        for call in blk["calls"]:
            g = call["g"]
            for w in blk["ws"]:
                for c in range(chunks_wg[w][g]):
                    jid = blk["chunk0"] + col
                    chunk_id[(w, g, c)] = jid
                    chunk_pos[jid] = call["pos0"] + (col - call["col0"]) * P
                    col += 1

    mdt = ml_dtypes
    pos_row = np.zeros((NC, n_pos), dtype=np.int16)    # gather idx per position
    masks = np.zeros((NC, P, n_chunks * W), dtype=mdt.bfloat16)
    for r in range(NC):
        m = e_owner == r
        dl = dloc[m]
        gr = grow[m]
        gg = ggrp[m]
        order = np.lexsort((gr, gg, dl // W))   # by (window, group), stable-ish
        dl, gr, gg = dl[order], gr[order], gg[order]
        wv = dl // W
        # boundaries of (w, g) runs
        keys = wv * NGRP + gg
        bounds = np.searchsorted(keys, np.arange(NW * NGRP + 1))
        for w in range(NW):
            for g in range(NGRP):
                lo, hi = bounds[w * NGRP + g], bounds[w * NGRP + g + 1]
                ne = hi - lo
                if ne == 0:
                    continue
                rel = (gr[lo:hi] - g * GRP).astype(np.int16)
                off = (dl[lo:hi] - w * W).astype(np.int64)
                s = np.arange(ne)
                for c in range(int(chunks_wg[w][g])):
                    jid = chunk_id[(w, g, c)]
                    sel = (s // P) == c
                    if not sel.any():
                        continue
                    sl = s[sel] % P
                    pos_row[r, chunk_pos[jid] + sl] = rel[sel]
                    masks[r, sl, jid * W + off[sel]] = 1.0

    # wrap indices: position i -> [i%16 (+16k replicas), i//16]
    idxw = np.zeros((NC, P, n_pos // 16), dtype=np.int16)
    for r in range(NC):
        a = pos_row[r].reshape(n_pos // 16, 16).T
        idxw[r] = np.tile(a, (8, 1))

    def to_pq(v):
        return np.ascontiguousarray(v.reshape(Q, P).T).astype(np.float32)

    cvec = np.zeros((NC, P, 3 * Q), dtype=np.float32)
    for r in range(NC):
        sl = slice(r * SH, (r + 1) * SH)
        c = np.zeros(SHP); c[:SH] = (1 - ALPHA) * no[sl] * ni[sl]
        cp = np.zeros(SHP); cp[:SH] = (1 - ALPHA) * ni[sl]
        nov = np.zeros(SHP); nov[:SH] = no[sl]
        cvec[r, :, :Q] = to_pq(c)
        cvec[r, :, Q:2 * Q] = to_pq(cp)
        cvec[r, :, 2 * Q:] = to_pq(nov)

    featsT = np.zeros((NC, IN_DIM, SHP), dtype=mdt.bfloat16)
    for r in range(NC):
        featsT[r, :, :SH] = feats[r * SH:(r + 1) * SH].T

    KB1 = IN_DIM // P
    MB1 = HID_DIM // P
    w1p = np.ascontiguousarray(
        W1.reshape(KB1, P, MB1, P).transpose(1, 0, 2, 3).reshape(P, KB1 * MB1 * P)
    ).astype(mdt.bfloat16)
    w2p = np.ascontiguousarray(
        W2.reshape(MB1, P, OUT_DIM).transpose(1, 0, 2).reshape(P, MB1 * OUT_DIM)
    ).astype(mdt.bfloat16)
    b1p = np.ascontiguousarray(b1.reshape(MB1, P).T).astype(np.float32)
    b2t = np.broadcast_to(b2, (P, OUT_DIM)).astype(np.float32).copy()

    in_maps = []
    for r in range(NC):
        in_maps.append(dict(
            featsT=np.ascontiguousarray(featsT[r]),
            w1p=w1p, w2p=w2p, b1p=b1p, b2t=b2t,
            cvec=np.ascontiguousarray(cvec[r]),
            idxw=np.ascontiguousarray(idxw[r]),
            masks=np.ascontiguousarray(masks[r]),
        ))
    meta = dict(n_chunks=n_chunks, n_pos=n_pos, blocks=blocks)
    return in_maps, meta


# ======================================================================
# device kernel builder
# ======================================================================
def build_kernel(n_chunks, n_pos, blocks):
    nc = bacc.Bacc("TRN2", target_bir_lowering=False, debug=False,
                   num_devices=NC)

    featsT = nc.dram_tensor("featsT", [IN_DIM, SHP], BF16, kind="ExternalInput")
    w1p = nc.dram_tensor("w1p", [P, (IN_DIM // P) * (HID_DIM // P) * P], BF16,
                         kind="ExternalInput")
    w2p = nc.dram_tensor("w2p", [P, (HID_DIM // P) * OUT_DIM], BF16,
                         kind="ExternalInput")
    b1p = nc.dram_tensor("b1p", [P, HID_DIM // P], F32, kind="ExternalInput")
    b2t = nc.dram_tensor("b2t", [P, OUT_DIM], F32, kind="ExternalInput")
    cvec_d = nc.dram_tensor("cvec", [P, 3 * Q], F32, kind="ExternalInput")
    idxw_d = nc.dram_tensor("idxw", [P, n_pos // 16], I16, kind="ExternalInput")
    masks_d = nc.dram_tensor("masks", [P, n_chunks * W], BF16, kind="ExternalInput")
    out_d = nc.dram_tensor("out", [P, Q * OUT_DIM], F32, kind="ExternalOutput")

    KB1 = IN_DIM // P    # 4
    MB1 = HID_DIM // P   # 2
    rblocks = [(i * 512, min(512, SHP - i * 512)) for i in range((SHP + 511) // 512)]
    cmax = max(blk["nchunks"] for blk in blocks)

    with tile.TileContext(nc) as tc:
        with tc.tile_pool(name="res", bufs=1) as res, \
             tc.tile_pool(name="dram", bufs=1, space="DRAM") as dram:

            # ---------- resident tiles ----------
            w1_sb = res.tile([P, KB1 * MB1 * P], BF16)
            nc.sync.dma_start(out=w1_sb[:], in_=w1p[:, :])
            w2_sb = res.tile([P, MB1 * OUT_DIM], BF16)
            nc.sync.dma_start(out=w2_sb[:], in_=w2p[:, :])
            b1_sb = res.tile([P, MB1], F32)
            nc.sync.dma_start(out=b1_sb[:], in_=b1p[:, :])
            b2_sb = res.tile([P, OUT_DIM], F32)
            nc.sync.dma_start(out=b2_sb[:], in_=b2t[:, :])
            cv_sb = res.tile([P, 3 * Q], F32)
            nc.sync.dma_start(out=cv_sb[:], in_=cvec_d[:, :])

            h0_sb = res.tile([P, Q * OUT_DIM], BF16)      # h0 in [p, q, f]
            g0_sb = res.tile([P, Q * ROW], BF16)          # g0 (+bvec in cols 64:112)
            nc.vector.memset(g0_sb[:], 0.0)

            # state staging for hops 1..K-1 (written by combine, DMA'd to cc_in)
            gst_sb = res.tile([P, Q * ROW], BF16)
            nc.vector.memset(gst_sb[:], 0.0)

            cc_in = [dram.tile([P, Q * ROW], BF16, name=f"ccin{i}")
                     for i in range(2)]
            gfull = [dram.tile([NROWS, ROW], BF16, name=f"gfull{i}",
                               addr_space="Shared") for i in range(K_HOPS)]

            # ---------- MLP ----------
            with tc.tile_pool(name="mlp", bufs=3) as mlp, \
                 tc.tile_pool(name="h1pool", bufs=1) as h1pool, \
                 tc.tile_pool(name="mpsum", bufs=2, space="PSUM") as mpsum:
                h1_sb = [h1pool.tile([P, SHP], BF16, name=f"h1_{m}")
                         for m in range(MB1)]
                for r0, rn in rblocks:
                    fts = []
                    for k in range(KB1):
                        ft = mlp.tile([P, 512], BF16, tag="ft", bufs=3)
                        nc.sync.dma_start(out=ft[:, :rn],
                                          in_=featsT[k * P:(k + 1) * P, r0:r0 + rn])
                        fts.append(ft)
                    for m in range(MB1):
                        ps1 = mpsum.tile([P, 512], F32, tag="ps1")
                        for k in range(KB1):
                            nc.tensor.matmul(
                                out=ps1[:, :rn],
                                lhsT=w1_sb[:, (k * MB1 + m) * P:(k * MB1 + m + 1) * P],
                                rhs=fts[k][:, :rn],
                                start=(k == 0), stop=(k == KB1 - 1))
                        nc.scalar.activation(
                            out=h1_sb[m][:, r0:r0 + rn], in_=ps1[:, :rn],
                            func=mybir.ActivationFunctionType.Relu,
                            bias=b1_sb[:, m:m + 1], scale=1.0)

                for q in range(Q):
                    ps2 = mpsum.tile([P, OUT_DIM], F32, tag="ps2")
                    for k in range(MB1):
                        nc.tensor.matmul(
                            out=ps2[:],
                            lhsT=h1_sb[k][:, q * P:(q + 1) * P],
                            rhs=w2_sb[:, k * OUT_DIM:(k + 1) * OUT_DIM],
                            start=(k == 0), stop=(k == MB1 - 1))
                    qs = slice(q * OUT_DIM, (q + 1) * OUT_DIM)
                    rs = slice(q * ROW, q * ROW + OUT_DIM)
                    bs = slice(q * ROW + 64, q * ROW + 64 + OUT_DIM)
                    nc.vector.tensor_add(out=h0_sb[:, qs], in0=ps2[:], in1=b2_sb[:])
                    nc.vector.tensor_scalar_mul(
                        out=g0_sb[:, rs], in0=h0_sb[:, qs],
                        scalar1=cv_sb[:, 2 * Q + q:2 * Q + q + 1])
                    nc.vector.tensor_scalar_mul(
                        out=g0_sb[:, bs], in0=g0_sb[:, rs], scalar1=ALPHA)

            # ---------- propagation hops ----------
            with tc.tile_pool(name="prop", bufs=2) as prop, \
                 tc.tile_pool(name="qpsum", bufs=4, space="PSUM") as qpsum, \
                 tc.tile_pool(name="tpool", bufs=4) as tpool:

                src_sb = g0_sb
                for t in range(K_HOPS):
                    ccb, gfb = cc_in[t % 2], gfull[t]
                    nc.sync.dma_start(out=ccb[:], in_=src_sb[:])
                    nc.gpsimd.collective_compute(
                        "AllGather", mybir.AluOpType.bypass,
                        replica_groups=[list(range(NC))],
                        ins=[ccb[:].opt()], outs=[gfb[:].opt()])

                    last = t == K_HOPS - 1
                    if last:
                        out_sb = prop.tile([P, Q * OUT_DIM], F32, tag="msg",
                                           bufs=2, name="out_sb")

                    for blk in blocks:
                        nch = blk["nchunks"]
                        msg = prop.tile([P, cmax * ROW], BF16, tag="msg", bufs=2,
                                        name=f"msg{t}_{blk['chunk0']}")
                        mk = prop.tile([P, cmax * W], BF16, tag="mk", bufs=2,
                                       name=f"mk{t}_{blk['chunk0']}")
                        idxs = prop.tile([P, (cmax * P) // 16], I16, tag="idx",
                                         bufs=2, name=f"idx{t}_{blk['chunk0']}")
                        nc.sync.dma_start(
                            out=mk[:, :nch * W],
                            in_=masks_d[:, blk["chunk0"] * W:(blk["chunk0"] + nch) * W])
                        nc.sync.dma_start(
                            out=idxs[:, :blk["npos"] // 16],
                            in_=idxw_d[:, blk["pos0"] // 16:
                                       (blk["pos0"] + blk["npos"]) // 16])
                        for call in blk["calls"]:
                            g = call["g"]
                            gsz = min(GRP, NROWS - g * GRP)
                            for s0 in range(0, call["npos"], MAX_CALL):
                                sn = min(MAX_CALL, call["npos"] - s0)
                                o0 = call["col0"] * ROW + (s0 // P) * ROW
                                i0 = (call["pos0"] - blk["pos0"] + s0) // 16
                                nc.gpsimd.dma_gather(
                                    out_ap=msg[:, o0:o0 + (sn // P) * ROW].rearrange(
                                        "p (c e) -> p c e", e=ROW),
                                    in_ap=gfb[g * GRP:g * GRP + gsz, :],
                                    idxs_ap=idxs[:, i0:i0 + sn // 16],
                                    num_idxs=sn,
                                    num_idxs_reg=sn,
                                    elem_size=ROW,
                                    single_packet=False,
                                )
                        cur_psum = None
                        for mm in blk["pe"]:
                            w = mm["w"]
                            q, wh = w // 2, w % 2
                            if wh == 0 and mm["start"]:
                                cur_psum = qpsum.tile([P, OUT_DIM], F32, tag="qp",
                                                      name=f"qp{t}_{q}")
                            col = mm["col"]
                            nc.tensor.matmul(
                                out=cur_psum[wh * W:(wh + 1) * W, :],
                                lhsT=mk[:, col * W:(col + 1) * W],
                                rhs=msg[:, col * ROW:col * ROW + OUT_DIM],
                                start=mm["start"], stop=mm["stop"],
                                tile_position=(0, wh * W),
                                skip_group_check=True)
                            if wh == 1 and mm["stop"]:
                                # quad complete -> combine
                                qs = slice(q * OUT_DIM, (q + 1) * OUT_DIM)
                                rs = slice(q * ROW, q * ROW + OUT_DIM)
                                bs = slice(q * ROW + 64, q * ROW + 64 + OUT_DIM)
                                tmp = tpool.tile([P, OUT_DIM], F32, tag="tmp")
                                if not last:
                                    nc.vector.tensor_scalar_mul(
                                        out=tmp[:], in0=cur_psum[:],
                                        scalar1=cv_sb[:, q:q + 1])
                                    nc.vector.tensor_add(
                                        out=gst_sb[:, rs], in0=tmp[:],
                                        in1=g0_sb[:, bs])
                                else:
                                    nc.vector.tensor_scalar_mul(
                                        out=tmp[:], in0=cur_psum[:],
                                        scalar1=cv_sb[:, Q + q:Q + q + 1])
                                    tmp2 = tpool.tile([P, OUT_DIM], F32, tag="tmp2")
                                    nc.vector.tensor_scalar_mul(
                                        out=tmp2[:], in0=h0_sb[:, qs],
                                        scalar1=ALPHA)
                                    nc.vector.tensor_add(
                                        out=out_sb[:, qs], in0=tmp[:], in1=tmp2[:])
                    src_sb = gst_sb

                nc.sync.dma_start(out=out_d[:, :], in_=out_sb[:])

    nc.compile()
    return nc


# ======================================================================
# entry point
# ======================================================================
_LAST_RESULTS = None


def kernel(**inputs):
    global _LAST_RESULTS
    in_maps, meta = preprocess(**inputs)
    nc = build_kernel(meta["n_chunks"], meta["n_pos"], meta["blocks"])
    trace = os.environ.get("APPNP_TRACE", "1") != "0"
    res = bass_utils.run_bass_kernel_spmd(
        nc, in_maps, core_ids=list(range(NC)), trace=trace)
    _LAST_RESULTS = res
    out = np.zeros((N, OUT_DIM), dtype=np.float32)
    for r in range(NC):
        sh = res.results[r]["out"].reshape(P, Q, OUT_DIM).transpose(1, 0, 2)
        out[r * SH:(r + 1) * SH] = sh.reshape(SHP, OUT_DIM)[:SH]
    return out
